# revision 31
# baseline (speedup 1.0000x reference)
"""Trainium2 Bass kernel (v10) for nn_AttentionBlock — reassociated causal attention.

Reference (per batch b):
    qs[t,j]    = sum_i s[t,i] Q[h,i,j]
    Omega[t,u] = sum_j qs[t,j] s[u,j]       (causal: keep u <= t)
    es[u,i]    = sum_j E[h,i,j] s[u,j]
    r[t,i]     = sum_h sum_u Omega[t,u] es[u,i]

Reassociation: for full (below-diagonal) 128-token blocks,
    sum_{u in blk} Omega[t,u] es[u,i] = qs[t,:] @ (s[blk].T @ es[blk])
so r's off-diagonal part = qs[bt] @ Gsum(bt) with Gsum the PSUM-accumulated
prefix of G_uc = s[uc].T @ es[uc]; only diagonal 128x128 Omega blocks are
materialized, masked by a DVE multiply with a precomputed triangular mask.

v22 over v9: heads processed in PAIRS with concatenated rhs operands (es, G,
and diag-Omega matmuls run at N=512/N=256 over both heads — 422 vs 601 PE
instructions), r accumulated in SBUF f32 via per-chunk DVE adds (frees 4 PSUM
banks for deeper transient rotation and removes the end-of-kernel PSUM
drain; pairs 1-2 route the per-chunk add ScalarE->SBUF then sum on the
otherwise-idle GpSimd so the DVE queue never delays a Gsum cast; the last
pair keeps a single DVE add so the output-drain chain stays short), the
Gsum snapshot is consumed one bt-iteration late (software
pipeline; the r_off matmuls are emitted BEFORE the G-accumulate so the PE
FIFO transitively orders each snapshot read before the next accumulate —
Tile does not emit that WAR edge itself), all DRAM tensors are pre-swizzled
on the host into [128, contiguous] layouts (12 large 2KB+/partition-line
DMAs at full HBM bandwidth), a burst of dummy matmuls at t=0 warms the PE
HAM clock gate during the input-DMA ramp, and the last head-pair (which has
no next-pair prep) gets 9 of its own prep groups deferred into its body as
PE filler PLUS a "stripe" formulation (r_off(tb) = qs[tb] @ S[tb-1] + an
explicit full Omega[tb,tb-1] block) that consumes each Gsum snapshot two
iterations after capture — the copy latency can no longer stall the PE and
the HAM clock gate stays at 8/8 through the whole tail (an explicit
ldweights on the snapshot guards the read-before-next-accumulate order).

v24 over v22 (trace-driven): the DMA fabric is ~280KB/us packet-rate
limited and round-robins across in-flight DMAs, so flooding all 12 input
DMAs at t=0 made EVERYTHING (including the critical first operands) finish
together at ~12.9us. Re-cut: only the critical set (sT half 0 + pair-0 Q,
partition-split across the two HW queues = 4x64 packets) is issued
upfront; all bulk rides the GpSimd software queue FIFO behind a gate op
that fires when sT0 lands, in exact consumption order (ETp0, sT1, s,
Q[4:16] and ET[1:4] as 6KB-line single DMAs). ET is re-laid-out pair-major
so a pair's slice is one contiguous 2KB line per partition. Warmup dummies
cut 12->5 (real work starts ~9.6us). The final output chunk is
partition-split across sync+scalar to halve the post-last-matmul packet
tail. Learned the hard way: GpSimd (Pool) has NO PSUM access on TRN2, so
every PSUM evacuation must stay on DVE/ScalarE — those two run ~85% busy
through the body and are the binding resource in the last pair.

Gotcha encoded here: a PSUM accumulation group spanning N banks needs
start=True on EACH bank's first matmul (start clears has_written for one
bank only).

Distribution: data-parallel over batch (8 batches = 8 cores, no collectives).
All matmuls bf16; f32 PSUM accumulation. Note: back-to-back benchmarking
runs heat the part into a lower power state (~+20% exec time); space
measurement runs out.
"""

import numpy as np
import ml_dtypes

import concourse.bacc as bacc
import concourse.mybir as mybir
import concourse.tile as tile
from concourse.bass_utils import run_bass_kernel_spmd

B = 8      # batch (== number of cores)
T = 1024   # tokens
NF = 256   # feature dim n
H = 8      # heads
P = 128    # partitions
TB = T // P    # 8 token blocks
JC = NF // P   # 2 feature chunks
NPAIR = H // 2
NCORES = 8

F32 = mybir.dt.float32
BF16 = mybir.dt.bfloat16
IS_GE = mybir.AluOpType.is_ge


def _emit(tc, nc, in0_d, in2_d, out_d, ctx):
    res = ctx.enter_context(tc.tile_pool(name="res", bufs=1))
    work = ctx.enter_context(tc.tile_pool(name="work", bufs=2))
    snap = ctx.enter_context(tc.tile_pool(name="snap", bufs=3))
    prp = ctx.enter_context(tc.tile_pool(name="prp", bufs=1, space="PSUM"))
    pgp = ctx.enter_context(tc.tile_pool(name="pgp", bufs=1, space="PSUM"))
    pwp = ctx.enter_context(tc.tile_pool(name="pwp", bufs=2, space="PSUM"))
    pdp = ctx.enter_context(tc.tile_pool(name="pdp", bufs=3, space="PSUM"))

    # Two host-packed input tensors: descriptor GENERATION is the serial
    # DMA resource (~69 desc/us, one per dst partition line, near-independent
    # of line size), so the whole first wave rides ONE 128-descriptor DMA
    # with 8KB lines (in0 = sT | Q[0:4] | ET pair0, done ~10.4us) and the
    # bulk rides a second with 16KB lines (in2 = s | Q[4:16] | ET pairs 1-3,
    # done ~16us). Partition-split or per-tensor DMAs only multiply
    # descriptor count (a 64-partition half runs at HALF rate).
    in0_sb = res.tile([P, 4096], BF16)   # [ sT(2048) | Qp0(1024) | ETp0(1024) ]
    in2_sb = res.tile([P, 8192], BF16)   # [ s(2048) | Q[4:16](3072) | ET[1:4](3072) ]
    mask4 = res.tile([P, 4, P], BF16)    # [u, (bt%2, h), t]: 1 where u <= t
    warm = res.tile([P, 640], BF16)
    r_sb = res.tile([P, TB, NF], F32)    # final r accumulator (SBUF)

    HP = P // 2
    nc.scalar.dma_start(out=in0_sb, in_=in0_d)
    nc.sync.dma_start(out=in2_sb, in_=in2_d)

    # views into the packed tiles (replacing the old sT_sb/Q_sb/ET_sb/s_sb)
    def sTv(tcx, jc, lo=0, hi=512):
        base = (tcx * 2 + jc) * 512
        return in0_sb[:, base + lo:base + hi]

    def Qv(c, lo=0, hi=NF):
        if c < 4:
            return in0_sb[:, 2048 + c * NF + lo:2048 + c * NF + hi]
        return in2_sb[:, 2048 + (c - 4) * NF + lo:2048 + (c - 4) * NF + hi]

    def ETv(p, jc):
        if p == 0:
            base = 3072 + jc * 512
            return in0_sb[:, base:base + 512]
        base = 5120 + (p - 1) * 1024 + jc * 512
        return in2_sb[:, base:base + 512]

    def sv(uc, jc):
        base = uc * NF + jc * P
        return in2_sb[:, base:base + P]

    # warm tile zeroed on the (idle) vector engine so warmup matmuls are not
    # queued behind gpsimd's DMA issues; mask setup stays on gpsimd.
    nc.vector.memset(warm, 0.0)
    nc.gpsimd.memset(mask4, 1.0)
    nc.gpsimd.affine_select(
        out=mask4, in_=mask4,
        pattern=[[0, 4], [1, P]],
        compare_op=IS_GE,   # keep 1.0 where t - u >= 0, else 0
        fill=0.0, base=0, channel_multiplier=-1,
    )

    # HAM warmup: dummy matmuls on a zeroed tile while input DMAs run, so the
    # PE clock gate is at 8/8 by the time real matmuls start.
    pwarm = pdp.tile([P, 512], F32, tag="pwd", name="pwarm")
    for _ in range(6):
        nc.tensor.matmul(pwarm, lhsT=warm[:, 0:128], rhs=warm[:, 128:640],
                         start=True, stop=True, skip_group_check=True)

    # Prep evacuations alternate VectorE/ScalarE.
    movers = [nc.vector.tensor_copy, nc.scalar.copy]
    mv = [0]

    def mover(out, in_, alt=True):
        movers[mv[0] % 2](out=out, in_=in_)
        mv[0] += 1

    # ---- per-pair prep: qsT for both heads and pair-concatenated es
    def prep_groups(p, pool_cycle=None, alt_from=None):
        h0 = 2 * p
        qsT2 = work.tile([P, 2, JC, T], BF16, tag="qsT", name=f"qsT{p}")
        es2 = work.tile([P, TB, 2 * NF], BF16, tag="es", name=f"es{p}")
        base_alt = pool_cycle is not None
        pool_cycle = pool_cycle or [(pwp, "pw")]

        def qsT_group(hh, jc, tcx, pool, tag, alt):
            pw = pool.tile([P, 512], F32, tag=tag, name="pwq")
            for ic in range(JC):
                nc.tensor.matmul(
                    pw,
                    lhsT=Qv((h0 + hh) * 2 + ic, jc * P, (jc + 1) * P),
                    rhs=sTv(tcx, ic),
                    start=(ic == 0), stop=(ic == JC - 1),
                    skip_group_check=True,
                )
            mover(qsT2[:, hh, jc, tcx * 512:(tcx + 1) * 512], pw, alt=alt)

        def es_group(uc, pool, tag, alt):
            pw = pool.tile([P, 512], F32, tag=tag, name="pwe")
            for jc in range(JC):
                nc.tensor.matmul(
                    pw,
                    lhsT=sTv(uc // 4, jc, (uc % 4) * P, (uc % 4 + 1) * P),
                    rhs=ETv(p, jc),
                    start=(jc == 0), stop=(jc == JC - 1),
                    skip_group_check=True,
                )
            mover(es2[:, uc, :], pw, alt=alt)

        # Order groups progressively: tcx0-qsT and low-uc es first (their
        # DMAs land first in the prologue); the tail of the list is safe to
        # defer into the consuming pair's own body (late-uc es / tcx1 qsT).
        specs = []
        for hh in range(2):
            for jc in range(JC):
                specs.append(("q", (hh, jc, 0)))
        for uc in range(3):
            specs.append(("e", (uc,)))
        for hh in range(2):
            for jc in range(JC):
                specs.append(("q", (hh, jc, 1)))
        for uc in range(3, TB):
            specs.append(("e", (uc,)))
        thunks = []
        for i, (kind, args) in enumerate(specs):
            pool, tag = pool_cycle[i % len(pool_cycle)]
            alt = base_alt or (alt_from is not None and i >= alt_from)
            if kind == "q":
                thunks.append(
                    lambda a=args, pool=pool, tag=tag, alt=alt:
                    qsT_group(*a, pool, tag, alt))
            else:
                thunks.append(
                    lambda a=args, pool=pool, tag=tag, alt=alt:
                    es_group(*a, pool, tag, alt))
        return qsT2, es2, thunks

    def body(p, qsT2, es2, nthunks, drain, drate=2, stripes=False):
        # stripes=True (last pair): r_off(tb) = qs[tb] @ S[tb-1] + explicit
        # Omega[tb, tb-1] stripe, so each Gsum snapshot is consumed TWO
        # iterations after it is taken — the PSUM->SBUF cast latency can
        # never stall the PE even with no next-pair prep to hide it behind.
        pg2 = pgp.tile([P, JC, 512], F32, tag="pg", name=f"pg{p}")
        gs_prev = None
        gs_prev2 = None
        gs = None
        pwd = None
        omd = None
        stro = None
        stro_prev = None
        rp = [None] * 4        # rp chunk tiles, one per 2-bt
        rp_started = [False] * 4
        for bt in range(TB + 1):
            if bt < TB:
                # [A] diag OmegaT block for both heads: [u, (h, t)]
                if bt % 2 == 0:
                    pwd = pdp.tile([P, 4, P], F32, tag="pwd", name="pwd")
                for jc in range(JC):
                    nc.tensor.matmul(
                        pwd[:, 2 * (bt % 2):2 * (bt % 2) + 2, :],
                        lhsT=sTv(bt // 4, jc, (bt % 4) * P, (bt % 4 + 1) * P),
                        rhs=qsT2[:, :, jc, bt * P:(bt + 1) * P],
                        start=(bt % 2 == 0 and jc == 0),
                        stop=(bt % 2 == 1 and jc == JC - 1),
                        skip_group_check=True,
                    )
                if stripes and bt >= 1:
                    # stripe OmegaT[u in bt-1, (h, t in bt)] (full block, no
                    # mask); consumed by [G] next iteration
                    pstro = pdp.tile([P, 4, P], F32, tag="pwd", name="pstro")
                    for jc in range(JC):
                        nc.tensor.matmul(
                            pstro[:, 0:2, :],
                            lhsT=sTv((bt - 1) // 4, jc, ((bt - 1) % 4) * P,
                                     ((bt - 1) % 4 + 1) * P),
                            rhs=qsT2[:, :, jc, bt * P:(bt + 1) * P],
                            start=(jc == 0), stop=(jc == JC - 1),
                            skip_group_check=True,
                        )
                    stro = snap.tile([P, 2, P], BF16, tag="stro")
                    nc.scalar.copy(out=stro, in_=pstro[:, 0:2, :])
            # [G] r_off(bt-1) = qs[bt-1] @ Gsum[bt-1] via last iter's snapshot.
            # MUST be emitted before [B]: [G]'s wait on the gs copy transitively
            # (via the PE FIFO) keeps this iteration's pg2-accumulating matmuls
            # from racing ahead of last iteration's snapshot read.
            if bt >= 2:
                tb = bt - 1
                k = tb // 2
                if rp[k] is None:
                    rp[k] = prp.tile([P, 2, NF], F32, tag="rp", name=f"rp{k}")
                if stripes:
                    for hh in range(2):
                        nc.tensor.matmul(
                            rp[k][:, tb % 2, :],
                            lhsT=stro_prev[:, hh, :],
                            rhs=es2[:, tb - 1, hh * NF:(hh + 1) * NF],
                            start=(not rp_started[k]),
                            stop=(tb == 1 and hh == 1),
                            skip_group_check=True,
                        )
                        rp_started[k] = True
                if not stripes or tb >= 2:
                    gsrc = gs_prev2 if stripes else gs_prev
                    for hh in range(2):
                        for jc in range(JC):
                            nc.tensor.matmul(
                                rp[k][:, tb % 2, :],
                                lhsT=qsT2[:, hh, jc, tb * P:(tb + 1) * P],
                                rhs=gsrc[:, jc, hh * NF:(hh + 1) * NF],
                                start=(not rp_started[k]),
                                stop=(tb % 2 == 1 and hh == 1 and jc == JC - 1),
                                skip_group_check=True,
                            )
                            rp_started[k] = True
            def emit_H(bt):
                k = (bt - 1) // 2
                sl = r_sb[:, 2 * k:2 * k + 2, :]
                if p == 0:
                    nc.scalar.copy(out=sl, in_=rp[k])
                elif not drain:
                    rps = snap.tile([P, 2, NF], F32, tag="rps")
                    nc.scalar.copy(out=rps, in_=rp[k])
                    nc.gpsimd.tensor_add(out=sl, in0=rps, in1=sl)
                else:
                    nc.vector.tensor_add(out=sl, in0=rp[k], in1=sl)
                if drain:
                    if k < 3:
                        nc.sync.dma_start(out=out_d[:, 2 * k:2 * k + 2, :],
                                          in_=sl)
                    else:
                        nc.sync.dma_start(out=out_d[0:HP, 6:8, :],
                                          in_=sl[0:HP])
                        nc.scalar.dma_start(out=out_d[HP:P, 6:8, :],
                                            in_=sl[HP:P])
            if drain and bt >= 2 and (bt - 1) % 2 == 1:
                emit_H(bt)
            if bt < TB:
                # [B] Gsum prefix accumulation + [C] snapshot. Under stripes
                # S[7] is never consumed: skip the last accumulate+snapshot.
                if bt >= 1 and not (stripes and bt == TB - 1):
                    uc = bt - 1
                    if stripes and gs_prev is not None:
                        # PE-FIFO guard: orders this iteration's accumulate
                        # after last iteration's snapshot read (Tile emits no
                        # WAR edge for mid-group PSUM reads).
                        nc.tensor.ldweights(weights=gs_prev[:, 0, 0:P])
                    for jc in range(JC):
                        nc.tensor.matmul(
                            pg2[:, jc, :],
                            lhsT=sv(uc, jc),
                            rhs=es2[:, uc, :],
                            # pg2 spans two banks (one per jc): each bank's
                            # first matmul needs start=True to clear its own
                            # has_written bits (start only clears ONE bank).
                            start=(bt == 1),
                            stop=(bt == (TB - 2 if stripes else TB - 1)
                                  and jc == JC - 1),
                            skip_group_check=True,
                        )
                    gs = snap.tile([P, JC, 512], BF16, tag="gs")
                    nc.vector.tensor_copy(out=gs[:, 0], in_=pg2[:, 0])
                    nc.scalar.copy(out=gs[:, 1], in_=pg2[:, 1])
            # [H] chunk complete -> accumulate into SBUF r, drain if last.
            # Pairs 1-2 route the add via ScalarE-evac + GpSimd (keeps the
            # DVE queue clear so gs casts land promptly); the drain pair's
            # DVE add was emitted BEFORE [B] (in emit_H after [G]) so it
            # precedes the gs cast in the DVE queue — its PSUM read frees
            # the single rp bank before the next chunk's matmuls need it.
            if (not drain) and bt >= 2 and (bt - 1) % 2 == 1:
                emit_H(bt)
            if bt < TB:
                # [D] prep groups, interleaved as PE filler
                for _ in range(drate):
                    if nthunks:
                        nthunks.pop(0)()
                # [E]+[F] mask the diag pair, then its r contribution
                if bt % 2 == 1:
                    omd = snap.tile([P, 4, P], BF16, tag="omd")
                    nc.vector.tensor_mul(omd, pwd, mask4)
                    for b2 in (bt - 1, bt):
                        k = b2 // 2
                        if rp[k] is None:
                            rp[k] = prp.tile([P, 2, NF], F32, tag="rp",
                                             name=f"rp{k}")
                        for hh in range(2):
                            nc.tensor.matmul(
                                rp[k][:, b2 % 2, :],
                                lhsT=omd[:, 2 * (b2 % 2) + hh, :],
                                rhs=es2[:, b2, hh * NF:(hh + 1) * NF],
                                start=(not rp_started[k]),
                                stop=False,
                                skip_group_check=True,
                            )
                            rp_started[k] = True
            gs_prev2 = gs_prev
            gs_prev = gs
            stro_prev = stro

    # pair-0 prep runs bare during the DMA ramp; rotate over all three
    # transient PSUM pools so evacuation latency never blocks the PE. The
    # deferred bulk DMAs are emitted between groups so the scalar/gpsimd
    # queues enqueue them only once the critical transfers are in flight.
    qsT2, es2, thunks = prep_groups(
        0, pool_cycle=[(pwp, "pw"), (pdp, "pwd"), (prp, "rp")])
    for th in thunks:
        th()
    # All of pair-3's prep runs during pair 2 (drate 2): pair 3 is
    # evacuation-bound on DVE/ScalarE, so giving it extra prep-evac work as
    # "PE filler" starved the Gsum casts; the stripe formulation alone gives
    # the 2-iteration slack its PE needs.
    for p in range(NPAIR):
        if p + 1 < NPAIR:
            nqsT2, nes2, nthunks = prep_groups(
                p + 1, alt_from=(7 if p + 1 == NPAIR - 1 else None))
        else:
            nqsT2, nes2, nthunks = None, None, []
        body(p, qsT2, es2, nthunks, drain=(p == NPAIR - 1), drate=2,
             stripes=(p == NPAIR - 1))
        for th in nthunks:   # any leftovers
            th()
        qsT2, es2 = nqsT2, nes2


def build():
    from contextlib import ExitStack

    nc = bacc.Bacc(
        "TRN2",
        target_bir_lowering=False,
        debug=False,
        enable_asserts=False,
        num_devices=NCORES,
    )
    in0_d = nc.dram_tensor("in0", [P, 4096], BF16, kind="ExternalInput").ap()
    in2_d = nc.dram_tensor("in2", [P, 8192], BF16, kind="ExternalInput").ap()
    out_d = nc.dram_tensor("out", [P, TB, NF], F32, kind="ExternalOutput").ap()
    with tile.TileContext(nc) as tc:
        with ExitStack() as ctx:
            _emit(tc, nc, in0_d, in2_d, out_d, ctx)
    nc.compile()
    return nc


_NC = None


def _get_nc():
    global _NC
    if _NC is None:
        _NC = build()
    return _NC


def _in_maps(s, Q, E):
    bf = ml_dtypes.bfloat16
    s = np.asarray(s, np.float32)
    Qf = np.asarray(Q, np.float32)
    Ef = np.asarray(E, np.float32)
    # Q_d[i1, h*2+ic, j] = Q[h, ic*128+i1, j]
    Qd = np.ascontiguousarray(
        Qf.reshape(H, JC, P, NF).transpose(2, 0, 1, 3).reshape(P, H * JC, NF)
    ).astype(bf)
    # ET_d[j1, pair, jc, hh, i] = E[2*pair+hh, i, jc*128+j1]
    ETd = np.ascontiguousarray(
        Ef.transpose(2, 0, 1)                  # [j, h, i]
        .reshape(JC, P, NPAIR, 2, NF)          # [jc, j1, pair, hh, i]
        .transpose(1, 2, 0, 3, 4)              # [j1, pair, jc, hh, i]
    ).astype(bf)
    maps = []
    for b in range(B):
        sb = s[b]
        sd = np.ascontiguousarray(
            sb.reshape(TB, P, NF).transpose(1, 0, 2)).astype(bf)
        sTd = np.ascontiguousarray(
            sb.T.reshape(JC, P, 2, 512).transpose(1, 2, 0, 3)).astype(bf)
        in0 = np.concatenate([
            sTd.reshape(P, 2048),
            Qd[:, 0:4].reshape(P, 1024),
            ETd[:, 0].reshape(P, 1024),
        ], axis=1)
        in2 = np.concatenate([
            sd.reshape(P, 2048),
            Qd[:, 4:16].reshape(P, 3072),
            ETd[:, 1:4].reshape(P, 3072),
        ], axis=1)
        maps.append({"in0": np.ascontiguousarray(in0),
                     "in2": np.ascontiguousarray(in2)})
    return maps


def _unpack(res):
    return np.stack([
        np.ascontiguousarray(
            res.results[b]["out"].transpose(1, 0, 2).reshape(T, NF))
        for b in range(B)], axis=0)


def kernel(s, Q, E):
    nc = _get_nc()
    res = run_bass_kernel_spmd(
        nc, _in_maps(s, Q, E), core_ids=list(range(NCORES)))
    return _unpack(res)


def run_profiled(s, Q, E, tmpdir=None):
    nc = _get_nc()
    res = run_bass_kernel_spmd(
        nc, _in_maps(s, Q, E), core_ids=list(range(NCORES)),
        trace=True, tmpdir=tmpdir)
    return _unpack(res), res.exec_time_ns



# revision 33
# speedup vs baseline: 1.0493x; 1.0493x over previous
"""Trainium2 Bass kernel (v10) for nn_AttentionBlock — reassociated causal attention.

Reference (per batch b):
    qs[t,j]    = sum_i s[t,i] Q[h,i,j]
    Omega[t,u] = sum_j qs[t,j] s[u,j]       (causal: keep u <= t)
    es[u,i]    = sum_j E[h,i,j] s[u,j]
    r[t,i]     = sum_h sum_u Omega[t,u] es[u,i]

Reassociation: for full (below-diagonal) 128-token blocks,
    sum_{u in blk} Omega[t,u] es[u,i] = qs[t,:] @ (s[blk].T @ es[blk])
so r's off-diagonal part = qs[bt] @ Gsum(bt) with Gsum the PSUM-accumulated
prefix of G_uc = s[uc].T @ es[uc]; only diagonal 128x128 Omega blocks are
materialized, masked by a DVE multiply with a precomputed triangular mask.

v22 over v9: heads processed in PAIRS with concatenated rhs operands (es, G,
and diag-Omega matmuls run at N=512/N=256 over both heads — 422 vs 601 PE
instructions), r accumulated in SBUF f32 via per-chunk DVE adds (frees 4 PSUM
banks for deeper transient rotation and removes the end-of-kernel PSUM
drain; pairs 1-2 route the per-chunk add ScalarE->SBUF then sum on the
otherwise-idle GpSimd so the DVE queue never delays a Gsum cast; the last
pair keeps a single DVE add so the output-drain chain stays short), the
Gsum snapshot is consumed one bt-iteration late (software
pipeline; the r_off matmuls are emitted BEFORE the G-accumulate so the PE
FIFO transitively orders each snapshot read before the next accumulate —
Tile does not emit that WAR edge itself), all DRAM tensors are pre-swizzled
on the host into [128, contiguous] layouts (12 large 2KB+/partition-line
DMAs at full HBM bandwidth), a burst of dummy matmuls at t=0 warms the PE
HAM clock gate during the input-DMA ramp, and the last head-pair (which has
no next-pair prep) gets 9 of its own prep groups deferred into its body as
PE filler PLUS a "stripe" formulation (r_off(tb) = qs[tb] @ S[tb-1] + an
explicit full Omega[tb,tb-1] block) that consumes each Gsum snapshot two
iterations after capture — the copy latency can no longer stall the PE and
the HAM clock gate stays at 8/8 through the whole tail (an explicit
ldweights on the snapshot guards the read-before-next-accumulate order).

v24 over v22 (trace-driven): the DMA fabric is ~280KB/us packet-rate
limited and round-robins across in-flight DMAs, so flooding all 12 input
DMAs at t=0 made EVERYTHING (including the critical first operands) finish
together at ~12.9us. Re-cut: only the critical set (sT half 0 + pair-0 Q,
partition-split across the two HW queues = 4x64 packets) is issued
upfront; all bulk rides the GpSimd software queue FIFO behind a gate op
that fires when sT0 lands, in exact consumption order (ETp0, sT1, s,
Q[4:16] and ET[1:4] as 6KB-line single DMAs). ET is re-laid-out pair-major
so a pair's slice is one contiguous 2KB line per partition. Warmup dummies
cut 12->5 (real work starts ~9.6us). The final output chunk is
partition-split across sync+scalar to halve the post-last-matmul packet
tail. Learned the hard way: GpSimd (Pool) has NO PSUM access on TRN2, so
every PSUM evacuation must stay on DVE/ScalarE — those two run ~85% busy
through the body and are the binding resource in the last pair.

Gotcha encoded here: a PSUM accumulation group spanning N banks needs
start=True on EACH bank's first matmul (start clears has_written for one
bank only).

Distribution: data-parallel over batch (8 batches = 8 cores, no collectives).
All matmuls bf16; f32 PSUM accumulation. Note: back-to-back benchmarking
runs heat the part into a lower power state (~+20% exec time); space
measurement runs out.
"""

import numpy as np
import ml_dtypes

import concourse.bacc as bacc
import concourse.mybir as mybir
import concourse.tile as tile
from concourse.bass_utils import run_bass_kernel_spmd

B = 8      # batch (== number of cores)
T = 1024   # tokens
NF = 256   # feature dim n
H = 8      # heads
P = 128    # partitions
TB = T // P    # 8 token blocks
JC = NF // P   # 2 feature chunks
NPAIR = H // 2
NCORES = 8

F32 = mybir.dt.float32
BF16 = mybir.dt.bfloat16
IS_GE = mybir.AluOpType.is_ge


def _emit(tc, nc, in0_d, inE_d, in2_d, out_d, ctx):
    res = ctx.enter_context(tc.tile_pool(name="res", bufs=1))
    work = ctx.enter_context(tc.tile_pool(name="work", bufs=2))
    snap = ctx.enter_context(tc.tile_pool(name="snap", bufs=3))
    # 8 PSUM banks exactly: rp double-buffered (the [H] add's PSUM read
    # must not WAR-block the next chunk's matmuls), Gsum 2, prep 2, diag 2
    prp = ctx.enter_context(tc.tile_pool(name="prp", bufs=2, space="PSUM"))
    pgp = ctx.enter_context(tc.tile_pool(name="pgp", bufs=1, space="PSUM"))
    pwp = ctx.enter_context(tc.tile_pool(name="pwp", bufs=2, space="PSUM"))
    pdp = ctx.enter_context(tc.tile_pool(name="pdp", bufs=2, space="PSUM"))

    # Two host-packed input tensors: descriptor GENERATION is the serial
    # DMA resource (~69 desc/us, one per dst partition line, near-independent
    # of line size), so the whole first wave rides ONE 128-descriptor DMA
    # with 8KB lines (in0 = sT | Q[0:4] | ET pair0, done ~10.4us) and the
    # bulk rides a second with 16KB lines (in2 = s | Q[4:16] | ET pairs 1-3,
    # done ~16us). Partition-split or per-tensor DMAs only multiply
    # descriptor count (a 64-partition half runs at HALF rate).
    in0_sb = res.tile([P, 3072], BF16)   # [ sT(2048) | Qp0(1024) ]
    inE_sb = res.tile([P, 1024], BF16)   # [ ETp0 ]
    in2_sb = res.tile([P, 8192], BF16)   # [ s(2048) | Q[4:16](3072) | ET[1:4](3072) ]
    mask4 = res.tile([P, 4, P], BF16)    # [u, (bt%2, h), t]: 1 where u <= t
    warm = res.tile([P, 640], BF16)
    r_sb = res.tile([P, TB, NF], F32)    # final r accumulator (SBUF)

    HP = P // 2
    # all three on the SCALAR queue: descriptor generation is one shared
    # serial unit across queues, so queue-FIFO = issue order guarantees the
    # critical 768KB rides alone first (done ~10.3), ETp0 next (~11.8),
    # the 2MB bulk last (~17) — each transfer overlapping the next descgen
    nc.scalar.dma_start(out=in0_sb, in_=in0_d)
    nc.scalar.dma_start(out=inE_sb, in_=inE_d)
    nc.scalar.dma_start(out=in2_sb, in_=in2_d)

    # views into the packed tiles (replacing the old sT_sb/Q_sb/ET_sb/s_sb)
    def sTv(tcx, jc, lo=0, hi=512):
        base = (tcx * 2 + jc) * 512
        return in0_sb[:, base + lo:base + hi]

    def Qv(c, lo=0, hi=NF):
        if c < 4:
            return in0_sb[:, 2048 + c * NF + lo:2048 + c * NF + hi]
        return in2_sb[:, 2048 + (c - 4) * NF + lo:2048 + (c - 4) * NF + hi]

    def ETv(p, jc):
        if p == 0:
            return inE_sb[:, jc * 512:jc * 512 + 512]
        base = 5120 + (p - 1) * 1024 + jc * 512
        return in2_sb[:, base:base + 512]

    def sv(uc, jc):
        base = uc * NF + jc * P
        return in2_sb[:, base:base + P]

    # warm tile zeroed on the (idle) vector engine so warmup matmuls are not
    # queued behind gpsimd's DMA issues; mask setup stays on gpsimd.
    nc.vector.memset(warm, 0.0)
    nc.gpsimd.memset(mask4, 1.0)
    nc.gpsimd.affine_select(
        out=mask4, in_=mask4,
        pattern=[[0, 4], [1, P]],
        compare_op=IS_GE,   # keep 1.0 where t - u >= 0, else 0
        fill=0.0, base=0, channel_multiplier=-1,
    )

    # HAM warmup: dummy matmuls on a zeroed tile while input DMAs run, so the
    # PE clock gate is at 8/8 by the time real matmuls start.
    pwarm = pdp.tile([P, 512], F32, tag="pwd", name="pwarm")
    for _ in range(6):
        nc.tensor.matmul(pwarm, lhsT=warm[:, 0:128], rhs=warm[:, 128:640],
                         start=True, stop=True, skip_group_check=True)

    # Prep evacuations alternate VectorE/ScalarE.
    movers = [nc.vector.tensor_copy, nc.scalar.copy]
    mv = [0]

    def mover(out, in_, alt=True):
        movers[mv[0] % 2](out=out, in_=in_)
        mv[0] += 1

    # ---- per-pair prep: qsT for both heads and pair-concatenated es
    def prep_groups(p, pool_cycle=None, alt_from=None):
        h0 = 2 * p
        qsT2 = work.tile([P, 2, JC, T], BF16, tag="qsT", name=f"qsT{p}")
        es2 = work.tile([P, TB, 2 * NF], BF16, tag="es", name=f"es{p}")
        base_alt = pool_cycle is not None
        pool_cycle = pool_cycle or [(pwp, "pw")]

        def qsT_group(hh, jc, tcx, pool, tag, alt):
            pw = pool.tile([P, 512], F32, tag=tag, name="pwq")
            for ic in range(JC):
                nc.tensor.matmul(
                    pw,
                    lhsT=Qv((h0 + hh) * 2 + ic, jc * P, (jc + 1) * P),
                    rhs=sTv(tcx, ic),
                    start=(ic == 0), stop=(ic == JC - 1),
                    skip_group_check=True,
                )
            mover(qsT2[:, hh, jc, tcx * 512:(tcx + 1) * 512], pw, alt=alt)

        def es_group(uc, pool, tag, alt):
            pw = pool.tile([P, 512], F32, tag=tag, name="pwe")
            for jc in range(JC):
                nc.tensor.matmul(
                    pw,
                    lhsT=sTv(uc // 4, jc, (uc % 4) * P, (uc % 4 + 1) * P),
                    rhs=ETv(p, jc),
                    start=(jc == 0), stop=(jc == JC - 1),
                    skip_group_check=True,
                )
            mover(es2[:, uc, :], pw, alt=alt)

        # Order groups progressively: tcx0-qsT and low-uc es first (their
        # DMAs land first in the prologue); the tail of the list is safe to
        # defer into the consuming pair's own body (late-uc es / tcx1 qsT).
        specs = []
        for hh in range(2):
            for jc in range(JC):
                specs.append(("q", (hh, jc, 0)))
        for uc in range(3):
            specs.append(("e", (uc,)))
        for hh in range(2):
            for jc in range(JC):
                specs.append(("q", (hh, jc, 1)))
        for uc in range(3, TB):
            specs.append(("e", (uc,)))
        thunks = []
        for i, (kind, args) in enumerate(specs):
            pool, tag = pool_cycle[i % len(pool_cycle)]
            alt = base_alt or (alt_from is not None and i >= alt_from)
            if kind == "q":
                thunks.append(
                    lambda a=args, pool=pool, tag=tag, alt=alt:
                    qsT_group(*a, pool, tag, alt))
            else:
                thunks.append(
                    lambda a=args, pool=pool, tag=tag, alt=alt:
                    es_group(*a, pool, tag, alt))
        return qsT2, es2, thunks

    def body(p, qsT2, es2, nthunks, drain, drate=2, stripes=False):
        # stripes=True (last pair): r_off(tb) = qs[tb] @ S[tb-1] + explicit
        # Omega[tb, tb-1] stripe, so each Gsum snapshot is consumed TWO
        # iterations after it is taken — the PSUM->SBUF cast latency can
        # never stall the PE even with no next-pair prep to hide it behind.
        pg2 = pgp.tile([P, JC, 512], F32, tag="pg", name=f"pg{p}")
        gs_prev = None
        gs_prev2 = None
        gs = None
        pwd = None
        omd = None
        stro = None
        stro_prev = None
        rp = [None] * 4        # rp chunk tiles, one per 2-bt
        rp_started = [False] * 4
        for bt in range(TB + 1):
            if bt < TB:
                # [A] diag OmegaT block for both heads: [u, (h, t)]
                if bt % 2 == 0:
                    pwd = pdp.tile([P, 4, P], F32, tag="pwd", name="pwd")
                for jc in range(JC):
                    nc.tensor.matmul(
                        pwd[:, 2 * (bt % 2):2 * (bt % 2) + 2, :],
                        lhsT=sTv(bt // 4, jc, (bt % 4) * P, (bt % 4 + 1) * P),
                        rhs=qsT2[:, :, jc, bt * P:(bt + 1) * P],
                        start=(bt % 2 == 0 and jc == 0),
                        stop=(bt % 2 == 1 and jc == JC - 1),
                        skip_group_check=True,
                    )
                if stripes and bt >= 1:
                    # stripe OmegaT[u in bt-1, (h, t in bt)] (full block, no
                    # mask); consumed by [G] next iteration
                    pstro = pdp.tile([P, 4, P], F32, tag="pwd", name="pstro")
                    for jc in range(JC):
                        nc.tensor.matmul(
                            pstro[:, 0:2, :],
                            lhsT=sTv((bt - 1) // 4, jc, ((bt - 1) % 4) * P,
                                     ((bt - 1) % 4 + 1) * P),
                            rhs=qsT2[:, :, jc, bt * P:(bt + 1) * P],
                            start=(jc == 0), stop=(jc == JC - 1),
                            skip_group_check=True,
                        )
                    stro = snap.tile([P, 2, P], BF16, tag="stro")
                    nc.scalar.copy(out=stro, in_=pstro[:, 0:2, :])
            # [G] r_off(bt-1) = qs[bt-1] @ Gsum[bt-1] via last iter's snapshot.
            # MUST be emitted before [B]: [G]'s wait on the gs copy transitively
            # (via the PE FIFO) keeps this iteration's pg2-accumulating matmuls
            # from racing ahead of last iteration's snapshot read.
            if bt >= 2:
                tb = bt - 1
                k = tb // 2
                if rp[k] is None:
                    rp[k] = prp.tile([P, 2, NF], F32, tag="rp", name=f"rp{k}")
                if stripes:
                    for hh in range(2):
                        nc.tensor.matmul(
                            rp[k][:, tb % 2, :],
                            lhsT=stro_prev[:, hh, :],
                            rhs=es2[:, tb - 1, hh * NF:(hh + 1) * NF],
                            start=(not rp_started[k]),
                            stop=(tb == 1 and hh == 1),
                            skip_group_check=True,
                        )
                        rp_started[k] = True
                if not stripes or tb >= 2:
                    gsrc = gs_prev2 if stripes else gs_prev
                    for hh in range(2):
                        for jc in range(JC):
                            nc.tensor.matmul(
                                rp[k][:, tb % 2, :],
                                lhsT=qsT2[:, hh, jc, tb * P:(tb + 1) * P],
                                rhs=gsrc[jc][:, hh * NF:(hh + 1) * NF],
                                start=(not rp_started[k]),
                                stop=(tb % 2 == 1 and hh == 1 and jc == JC - 1),
                                skip_group_check=True,
                            )
                            rp_started[k] = True
            def emit_H(bt):
                k = (bt - 1) // 2
                sl = r_sb[:, 2 * k:2 * k + 2, :]
                if p == 0:
                    nc.scalar.copy(out=sl, in_=rp[k])
                elif not drain:
                    rps = snap.tile([P, 2, NF], F32, tag="rps")
                    nc.scalar.copy(out=rps, in_=rp[k])
                    nc.gpsimd.tensor_add(out=sl, in0=rps, in1=sl)
                else:
                    nc.vector.tensor_add(out=sl, in0=rp[k], in1=sl)
                if drain:
                    if k < 3:
                        nc.sync.dma_start(out=out_d[:, 2 * k:2 * k + 2, :],
                                          in_=sl)
                    else:
                        nc.sync.dma_start(out=out_d[0:HP, 6:8, :],
                                          in_=sl[0:HP])
                        nc.scalar.dma_start(out=out_d[HP:P, 6:8, :],
                                            in_=sl[HP:P])
            if drain and bt >= 2 and (bt - 1) % 2 == 1:
                emit_H(bt)
            if bt < TB:
                # [B] Gsum prefix accumulation + [C] snapshot. Under stripes
                # S[7] is never consumed: skip the last accumulate+snapshot.
                if bt >= 1 and not (stripes and bt == TB - 1):
                    uc = bt - 1
                    if stripes and gs_prev is not None:
                        # PE-FIFO guard: orders this iteration's accumulate
                        # after last iteration's snapshot read (Tile emits no
                        # WAR edge for mid-group PSUM reads).
                        nc.tensor.ldweights(weights=gs_prev[0][:, 0:P])
                    for jc in range(JC):
                        nc.tensor.matmul(
                            pg2[:, jc, :],
                            lhsT=sv(uc, jc),
                            rhs=es2[:, uc, :],
                            # pg2 spans two banks (one per jc): each bank's
                            # first matmul needs start=True to clear its own
                            # has_written bits (start only clears ONE bank).
                            start=(bt == 1),
                            stop=(bt == (TB - 2 if stripes else TB - 1)
                                  and jc == JC - 1),
                            skip_group_check=True,
                        )
                    # gs halves are SEPARATE tiles: a shared tile ties the
                    # DVE cast to the ScalarE cast of 3 snapshots ago via
                    # whole-tile WAW tracking, head-of-line blocking the DVE
                    # queue behind a busy ScalarE
                    gs = (snap.tile([P, 512], BF16, tag="gs0", name="gs0"),
                          snap.tile([P, 512], BF16, tag="gs1", name="gs1"))
                    nc.vector.tensor_copy(out=gs[0], in_=pg2[:, 0])
                    nc.scalar.copy(out=gs[1], in_=pg2[:, 1])
            # [H] chunk complete -> accumulate into SBUF r, drain if last.
            # Pairs 1-2 route the add via ScalarE-evac + GpSimd (keeps the
            # DVE queue clear so gs casts land promptly); the drain pair's
            # DVE add was emitted BEFORE [B] (in emit_H after [G]) so it
            # precedes the gs cast in the DVE queue — its PSUM read frees
            # the single rp bank before the next chunk's matmuls need it.
            if (not drain) and bt >= 2 and (bt - 1) % 2 == 1:
                emit_H(bt)
            if bt < TB:
                # [D] prep groups, interleaved as PE filler
                for _ in range(drate):
                    if nthunks:
                        nthunks.pop(0)()
                # [E]+[F] mask the diag pair, then its r contribution
                if bt % 2 == 1:
                    omd = snap.tile([P, 4, P], BF16, tag="omd")
                    nc.vector.tensor_mul(omd, pwd, mask4)
                    for b2 in (bt - 1, bt):
                        k = b2 // 2
                        if rp[k] is None:
                            rp[k] = prp.tile([P, 2, NF], F32, tag="rp",
                                             name=f"rp{k}")
                        for hh in range(2):
                            nc.tensor.matmul(
                                rp[k][:, b2 % 2, :],
                                lhsT=omd[:, 2 * (b2 % 2) + hh, :],
                                rhs=es2[:, b2, hh * NF:(hh + 1) * NF],
                                start=(not rp_started[k]),
                                stop=False,
                                skip_group_check=True,
                            )
                            rp_started[k] = True
            gs_prev2 = gs_prev
            gs_prev = gs
            stro_prev = stro

    # pair-0 prep runs bare during the DMA ramp; rotate over all three
    # transient PSUM pools so evacuation latency never blocks the PE. The
    # deferred bulk DMAs are emitted between groups so the scalar/gpsimd
    # queues enqueue them only once the critical transfers are in flight.
    qsT2, es2, thunks = prep_groups(
        0, pool_cycle=[(pwp, "pw"), (pdp, "pwd"), (prp, "rp")])
    for th in thunks:
        th()
    # All of pair-3's prep runs during pair 2 (drate 2): pair 3 is
    # evacuation-bound on DVE/ScalarE, so giving it extra prep-evac work as
    # "PE filler" starved the Gsum casts; the stripe formulation alone gives
    # the 2-iteration slack its PE needs.
    for p in range(NPAIR):
        if p + 1 < NPAIR:
            nqsT2, nes2, nthunks = prep_groups(
                p + 1, alt_from=(7 if p + 1 == NPAIR - 1 else None))
        else:
            nqsT2, nes2, nthunks = None, None, []
        body(p, qsT2, es2, nthunks, drain=(p == NPAIR - 1), drate=2,
             stripes=(p == NPAIR - 1))
        for th in nthunks:   # any leftovers
            th()
        qsT2, es2 = nqsT2, nes2


def build():
    from contextlib import ExitStack

    nc = bacc.Bacc(
        "TRN2",
        target_bir_lowering=False,
        debug=False,
        enable_asserts=False,
        num_devices=NCORES,
    )
    in0_d = nc.dram_tensor("in0", [P, 3072], BF16, kind="ExternalInput").ap()
    inE_d = nc.dram_tensor("inE", [P, 1024], BF16, kind="ExternalInput").ap()
    in2_d = nc.dram_tensor("in2", [P, 8192], BF16, kind="ExternalInput").ap()
    out_d = nc.dram_tensor("out", [P, TB, NF], F32, kind="ExternalOutput").ap()
    with tile.TileContext(nc) as tc:
        with ExitStack() as ctx:
            _emit(tc, nc, in0_d, inE_d, in2_d, out_d, ctx)
    nc.compile()
    return nc


_NC = None


def _get_nc():
    global _NC
    if _NC is None:
        _NC = build()
    return _NC


def _in_maps(s, Q, E):
    bf = ml_dtypes.bfloat16
    s = np.asarray(s, np.float32)
    Qf = np.asarray(Q, np.float32)
    Ef = np.asarray(E, np.float32)
    # Q_d[i1, h*2+ic, j] = Q[h, ic*128+i1, j]
    Qd = np.ascontiguousarray(
        Qf.reshape(H, JC, P, NF).transpose(2, 0, 1, 3).reshape(P, H * JC, NF)
    ).astype(bf)
    # ET_d[j1, pair, jc, hh, i] = E[2*pair+hh, i, jc*128+j1]
    ETd = np.ascontiguousarray(
        Ef.transpose(2, 0, 1)                  # [j, h, i]
        .reshape(JC, P, NPAIR, 2, NF)          # [jc, j1, pair, hh, i]
        .transpose(1, 2, 0, 3, 4)              # [j1, pair, jc, hh, i]
    ).astype(bf)
    maps = []
    for b in range(B):
        sb = s[b]
        sd = np.ascontiguousarray(
            sb.reshape(TB, P, NF).transpose(1, 0, 2)).astype(bf)
        sTd = np.ascontiguousarray(
            sb.T.reshape(JC, P, 2, 512).transpose(1, 2, 0, 3)).astype(bf)
        in0 = np.concatenate([
            sTd.reshape(P, 2048),
            Qd[:, 0:4].reshape(P, 1024),
        ], axis=1)
        in2 = np.concatenate([
            sd.reshape(P, 2048),
            Qd[:, 4:16].reshape(P, 3072),
            ETd[:, 1:4].reshape(P, 3072),
        ], axis=1)
        maps.append({"in0": np.ascontiguousarray(in0),
                     "inE": np.ascontiguousarray(ETd[:, 0].reshape(P, 1024)),
                     "in2": np.ascontiguousarray(in2)})
    return maps


def _unpack(res):
    return np.stack([
        np.ascontiguousarray(
            res.results[b]["out"].transpose(1, 0, 2).reshape(T, NF))
        for b in range(B)], axis=0)


def kernel(s, Q, E):
    nc = _get_nc()
    res = run_bass_kernel_spmd(
        nc, _in_maps(s, Q, E), core_ids=list(range(NCORES)))
    return _unpack(res)


def run_profiled(s, Q, E, tmpdir=None):
    nc = _get_nc()
    res = run_bass_kernel_spmd(
        nc, _in_maps(s, Q, E), core_ids=list(range(NCORES)),
        trace=True, tmpdir=tmpdir)
    return _unpack(res), res.exec_time_ns



# revision 36
# speedup vs baseline: 1.0997x; 1.0480x over previous
"""Trainium2 Bass kernel (v10) for nn_AttentionBlock — reassociated causal attention.

Reference (per batch b):
    qs[t,j]    = sum_i s[t,i] Q[h,i,j]
    Omega[t,u] = sum_j qs[t,j] s[u,j]       (causal: keep u <= t)
    es[u,i]    = sum_j E[h,i,j] s[u,j]
    r[t,i]     = sum_h sum_u Omega[t,u] es[u,i]

Reassociation: for full (below-diagonal) 128-token blocks,
    sum_{u in blk} Omega[t,u] es[u,i] = qs[t,:] @ (s[blk].T @ es[blk])
so r's off-diagonal part = qs[bt] @ Gsum(bt) with Gsum the PSUM-accumulated
prefix of G_uc = s[uc].T @ es[uc]; only diagonal 128x128 Omega blocks are
materialized, masked by a DVE multiply with a precomputed triangular mask.

v22 over v9: heads processed in PAIRS with concatenated rhs operands (es, G,
and diag-Omega matmuls run at N=512/N=256 over both heads — 422 vs 601 PE
instructions), r accumulated in SBUF f32 via per-chunk DVE adds (frees 4 PSUM
banks for deeper transient rotation and removes the end-of-kernel PSUM
drain; pairs 1-2 route the per-chunk add ScalarE->SBUF then sum on the
otherwise-idle GpSimd so the DVE queue never delays a Gsum cast; the last
pair keeps a single DVE add so the output-drain chain stays short), the
Gsum snapshot is consumed one bt-iteration late (software
pipeline; the r_off matmuls are emitted BEFORE the G-accumulate so the PE
FIFO transitively orders each snapshot read before the next accumulate —
Tile does not emit that WAR edge itself), all DRAM tensors are pre-swizzled
on the host into [128, contiguous] layouts (12 large 2KB+/partition-line
DMAs at full HBM bandwidth), a burst of dummy matmuls at t=0 warms the PE
HAM clock gate during the input-DMA ramp, and the last head-pair (which has
no next-pair prep) gets 9 of its own prep groups deferred into its body as
PE filler PLUS a "stripe" formulation (r_off(tb) = qs[tb] @ S[tb-1] + an
explicit full Omega[tb,tb-1] block) that consumes each Gsum snapshot two
iterations after capture — the copy latency can no longer stall the PE and
the HAM clock gate stays at 8/8 through the whole tail (an explicit
ldweights on the snapshot guards the read-before-next-accumulate order).

v31 over v22 (trace-driven; balanced A/B: 96.1us mean vs 97.9 for v22):
(1) DMA descriptor GENERATION is the serial fabric resource (~69 desc/us
shared across all queues, one descriptor per dst partition line,
near-independent of line size; per-core transfer bw ~400GB/s; partition-
split halves run at HALF rate) — so the 12-DMA input wave, whose critical
first operands used to finish with everything else at ~12.9us, is repacked
HOST-SIDE into three FIFO DMAs on one queue: [sT|Qp0] 768KB with 6KB lines
(done ~10.3us), [ETp0] (~11.8), then the 2MB bulk [s|Q4:16|ET1:4] with 16KB
lines (~17); each transfer overlaps the next DMA's descgen. Warmup dummies
12->6. (2) The Gsum snapshot halves are SEPARATE tiles: one shared tile
tied the DVE cast to the ScalarE cast of 3 snapshots ago via whole-tile
WAW tracking, head-of-line blocking the DVE queue. (3) The final output
chunk is partition-split across both HW queues, halving the post-last-
matmul descgen+transfer tail. Body schedule (head pairs, stripes, carry,
PSUM pools) is v22's unchanged — experiments that rebalanced it (carry
removal, prp/pdp rebanking, mask-mul offload) all lost more in new WAR/
queue stalls than they saved; GpSimd (Pool) has NO PSUM access on TRN2 so
every PSUM evacuation must share DVE+ScalarE, which run ~85% busy.

Gotcha encoded here: a PSUM accumulation group spanning N banks needs
start=True on EACH bank's first matmul (start clears has_written for one
bank only).

Distribution: data-parallel over batch (8 batches = 8 cores, no collectives).
All matmuls bf16; f32 PSUM accumulation. ~98.7us vs 111us baseline; rel err
3.07e-3. Note: back-to-back benchmarking runs heat the part into a lower
power state (~+20% exec time); space measurement runs out.
"""

import numpy as np
import ml_dtypes

import concourse.bacc as bacc
import concourse.mybir as mybir
import concourse.tile as tile
from concourse.bass_utils import run_bass_kernel_spmd

B = 8      # batch (== number of cores)
T = 1024   # tokens
NF = 256   # feature dim n
H = 8      # heads
P = 128    # partitions
TB = T // P    # 8 token blocks
JC = NF // P   # 2 feature chunks
NPAIR = H // 2
NCORES = 8

F32 = mybir.dt.float32
BF16 = mybir.dt.bfloat16
IS_GE = mybir.AluOpType.is_ge


def _emit(tc, nc, in0_d, inE_d, in2_d, out_d, ctx):
    res = ctx.enter_context(tc.tile_pool(name="res", bufs=1))
    work = ctx.enter_context(tc.tile_pool(name="work", bufs=2))
    snap = ctx.enter_context(tc.tile_pool(name="snap", bufs=3))
    prp = ctx.enter_context(tc.tile_pool(name="prp", bufs=1, space="PSUM"))
    pgp = ctx.enter_context(tc.tile_pool(name="pgp", bufs=1, space="PSUM"))
    pwp = ctx.enter_context(tc.tile_pool(name="pwp", bufs=2, space="PSUM"))
    pdp = ctx.enter_context(tc.tile_pool(name="pdp", bufs=3, space="PSUM"))

    # Host-packed inputs: DMA descriptor generation is the serial resource
    # (~69 desc/us shared across queues, one desc per dst partition line) and
    # per-core transfer bw is ~400GB/s, so the wave is three FIFO DMAs on one
    # queue: critical [sT|Qp0] 768KB (done ~10.3us), ETp0 (~11.8), then the
    # 2MB bulk [s|Q4:16|ET1:4] (~17) — each transfer overlaps the next
    # descgen. Partition-splits/per-tensor DMAs only multiply descriptors.
    in0_sb = res.tile([P, 3072], BF16)   # [ sT(2048) | Qp0(1024) ]
    inE_sb = res.tile([P, 1024], BF16)   # [ ETp0 ]
    in2_sb = res.tile([P, 8192], BF16)   # [ s(2048) | Q[4:16](3072) | ET[1:4](3072) ]
    mask4 = res.tile([P, 4, P], BF16)    # [u, (bt%2, h), t]: 1 where u <= t
    warm = res.tile([P, 640], BF16)
    r_sb = res.tile([P, TB, NF], F32)    # final r accumulator (SBUF)

    HP = P // 2
    nc.scalar.dma_start(out=in0_sb, in_=in0_d)
    nc.scalar.dma_start(out=inE_sb, in_=inE_d)
    nc.scalar.dma_start(out=in2_sb, in_=in2_d)

    # views into the packed tiles (old sT_sb / Q_sb / ET_sb / s_sb layouts)
    def sTv(tcx, jc, lo=0, hi=512):
        base = (tcx * 2 + jc) * 512
        return in0_sb[:, base + lo:base + hi]

    def Qv(c, lo=0, hi=NF):
        if c < 4:
            return in0_sb[:, 2048 + c * NF + lo:2048 + c * NF + hi]
        return in2_sb[:, 2048 + (c - 4) * NF + lo:2048 + (c - 4) * NF + hi]

    def ETv(p, jc):
        if p == 0:
            return inE_sb[:, jc * 512:jc * 512 + 512]
        base = 5120 + (p - 1) * 1024 + jc * 512
        return in2_sb[:, base:base + 512]

    def sv(uc, jc):
        base = uc * NF + jc * P
        return in2_sb[:, base:base + P]

    # warm tile zeroed on the (idle) vector engine so warmup matmuls are not
    # queued behind gpsimd's DMA issues; mask setup stays on gpsimd.
    nc.vector.memset(warm, 0.0)
    nc.gpsimd.memset(mask4, 1.0)
    nc.gpsimd.affine_select(
        out=mask4, in_=mask4,
        pattern=[[0, 4], [1, P]],
        compare_op=IS_GE,   # keep 1.0 where t - u >= 0, else 0
        fill=0.0, base=0, channel_multiplier=-1,
    )

    # HAM warmup: dummy matmuls on a zeroed tile while input DMAs run, so the
    # PE clock gate is at 8/8 by the time real matmuls start.
    pwarm = pdp.tile([P, 512], F32, tag="pwd", name="pwarm")
    for _ in range(6):
        nc.tensor.matmul(pwarm, lhsT=warm[:, 0:128], rhs=warm[:, 128:640],
                         start=True, stop=True, skip_group_check=True)

    # Prep evacuations alternate VectorE/ScalarE.
    movers = [nc.vector.tensor_copy, nc.scalar.copy]
    mv = [0]

    def mover(out, in_, alt=True):
        movers[mv[0] % 2](out=out, in_=in_)
        mv[0] += 1

    # ---- per-pair prep: qsT for both heads and pair-concatenated es
    def prep_groups(p, pool_cycle=None, alt_from=None):
        h0 = 2 * p
        qsT2 = work.tile([P, 2, JC, T], BF16, tag="qsT", name=f"qsT{p}")
        es2 = work.tile([P, TB, 2 * NF], BF16, tag="es", name=f"es{p}")
        base_alt = pool_cycle is not None
        pool_cycle = pool_cycle or [(pwp, "pw")]

        def qsT_group(hh, jc, tcx, pool, tag, alt):
            pw = pool.tile([P, 512], F32, tag=tag, name="pwq")
            for ic in range(JC):
                nc.tensor.matmul(
                    pw,
                    lhsT=Qv((h0 + hh) * 2 + ic, jc * P, (jc + 1) * P),
                    rhs=sTv(tcx, ic),
                    start=(ic == 0), stop=(ic == JC - 1),
                    skip_group_check=True,
                )
            mover(qsT2[:, hh, jc, tcx * 512:(tcx + 1) * 512], pw, alt=alt)

        def es_group(uc, pool, tag, alt):
            pw = pool.tile([P, 512], F32, tag=tag, name="pwe")
            for jc in range(JC):
                nc.tensor.matmul(
                    pw,
                    lhsT=sTv(uc // 4, jc, (uc % 4) * P, (uc % 4 + 1) * P),
                    rhs=ETv(h0 // 2, jc),
                    start=(jc == 0), stop=(jc == JC - 1),
                    skip_group_check=True,
                )
            mover(es2[:, uc, :], pw, alt=alt)

        # Order groups progressively: tcx0-qsT and low-uc es first (their
        # DMAs land first in the prologue); the tail of the list is safe to
        # defer into the consuming pair's own body (late-uc es / tcx1 qsT).
        specs = []
        for hh in range(2):
            for jc in range(JC):
                specs.append(("q", (hh, jc, 0)))
        for uc in range(3):
            specs.append(("e", (uc,)))
        for hh in range(2):
            for jc in range(JC):
                specs.append(("q", (hh, jc, 1)))
        for uc in range(3, TB):
            specs.append(("e", (uc,)))
        thunks = []
        for i, (kind, args) in enumerate(specs):
            pool, tag = pool_cycle[i % len(pool_cycle)]
            alt = base_alt or (alt_from is not None and i >= alt_from)
            if kind == "q":
                thunks.append(
                    lambda a=args, pool=pool, tag=tag, alt=alt:
                    qsT_group(*a, pool, tag, alt))
            else:
                thunks.append(
                    lambda a=args, pool=pool, tag=tag, alt=alt:
                    es_group(*a, pool, tag, alt))
        return qsT2, es2, thunks

    def body(p, qsT2, es2, nthunks, drain, drate=2, stripes=False):
        # stripes=True (last pair): r_off(tb) = qs[tb] @ S[tb-1] + explicit
        # Omega[tb, tb-1] stripe, so each Gsum snapshot is consumed TWO
        # iterations after it is taken — the PSUM->SBUF cast latency can
        # never stall the PE even with no next-pair prep to hide it behind.
        pg2 = pgp.tile([P, JC, 512], F32, tag="pg", name=f"pg{p}")
        gs_prev = None
        gs_prev2 = None
        gs = None
        pwd = None
        omd = None
        stro = None
        stro_prev = None
        rp = [None] * 4        # rp chunk tiles, one per 2-bt
        rp_started = [False] * 4
        for bt in range(TB + 1):
            if bt < TB:
                # [A] diag OmegaT block for both heads: [u, (h, t)]
                if bt % 2 == 0:
                    pwd = pdp.tile([P, 4, P], F32, tag="pwd", name="pwd")
                for jc in range(JC):
                    nc.tensor.matmul(
                        pwd[:, 2 * (bt % 2):2 * (bt % 2) + 2, :],
                        lhsT=sTv(bt // 4, jc, (bt % 4) * P, (bt % 4 + 1) * P),
                        rhs=qsT2[:, :, jc, bt * P:(bt + 1) * P],
                        start=(bt % 2 == 0 and jc == 0),
                        stop=(bt % 2 == 1 and jc == JC - 1),
                        skip_group_check=True,
                    )
                if stripes and bt >= 1:
                    # stripe OmegaT[u in bt-1, (h, t in bt)] (full block, no
                    # mask); consumed by [G] next iteration
                    pstro = pdp.tile([P, 4, P], F32, tag="pwd", name="pstro")
                    for jc in range(JC):
                        nc.tensor.matmul(
                            pstro[:, 0:2, :],
                            lhsT=sTv((bt - 1) // 4, jc, ((bt - 1) % 4) * P,
                                     ((bt - 1) % 4 + 1) * P),
                            rhs=qsT2[:, :, jc, bt * P:(bt + 1) * P],
                            start=(jc == 0), stop=(jc == JC - 1),
                            skip_group_check=True,
                        )
                    stro = snap.tile([P, 2, P], BF16, tag="stro")
                    nc.scalar.copy(out=stro, in_=pstro[:, 0:2, :])
            # [G] r_off(bt-1) = qs[bt-1] @ Gsum[bt-1] via last iter's snapshot.
            # MUST be emitted before [B]: [G]'s wait on the gs copy transitively
            # (via the PE FIFO) keeps this iteration's pg2-accumulating matmuls
            # from racing ahead of last iteration's snapshot read.
            if bt >= 2:
                tb = bt - 1
                k = tb // 2
                if rp[k] is None:
                    rp[k] = prp.tile([P, 2, NF], F32, tag="rp", name=f"rp{k}")
                if stripes:
                    for hh in range(2):
                        nc.tensor.matmul(
                            rp[k][:, tb % 2, :],
                            lhsT=stro_prev[:, hh, :],
                            rhs=es2[:, tb - 1, hh * NF:(hh + 1) * NF],
                            start=(not rp_started[k]),
                            stop=(tb == 1 and hh == 1),
                            skip_group_check=True,
                        )
                        rp_started[k] = True
                if not stripes or tb >= 2:
                    gsrc = gs_prev2 if stripes else gs_prev
                    for hh in range(2):
                        for jc in range(JC):
                            nc.tensor.matmul(
                                rp[k][:, tb % 2, :],
                                lhsT=qsT2[:, hh, jc, tb * P:(tb + 1) * P],
                                rhs=gsrc[jc][:, hh * NF:(hh + 1) * NF],
                                start=(not rp_started[k]),
                                stop=(tb % 2 == 1 and hh == 1 and jc == JC - 1),
                                skip_group_check=True,
                            )
                            rp_started[k] = True
            if bt < TB:
                # [B] Gsum prefix accumulation + [C] snapshot. Under stripes
                # S[7] is never consumed: skip the last accumulate+snapshot.
                if bt >= 1 and not (stripes and bt == TB - 1):
                    uc = bt - 1
                    if stripes and gs_prev is not None:
                        # PE-FIFO guard: orders this iteration's accumulate
                        # after last iteration's snapshot read (Tile emits no
                        # WAR edge for mid-group PSUM reads).
                        nc.tensor.ldweights(weights=gs_prev[0][:, 0:P])
                    for jc in range(JC):
                        nc.tensor.matmul(
                            pg2[:, jc, :],
                            lhsT=sv(uc, jc),
                            rhs=es2[:, uc, :],
                            # pg2 spans two banks (one per jc): each bank's
                            # first matmul needs start=True to clear its own
                            # has_written bits (start only clears ONE bank).
                            start=(bt == 1),
                            stop=(bt == (TB - 2 if stripes else TB - 1)
                                  and jc == JC - 1),
                            skip_group_check=True,
                        )
                    # gs halves are SEPARATE tiles: one shared tile ties
                    # the DVE cast to the ScalarE cast of 3 snapshots ago
                    # via whole-tile WAW tracking, head-of-line blocking the
                    # DVE queue behind a busy ScalarE
                    gs = (snap.tile([P, 512], BF16, tag="gs0", name="gs0"),
                          snap.tile([P, 512], BF16, tag="gs1", name="gs1"))
                    nc.vector.tensor_copy(out=gs[0], in_=pg2[:, 0])
                    nc.scalar.copy(out=gs[1], in_=pg2[:, 1])
            # [H] chunk complete -> accumulate into SBUF r, drain if last.
            # Pairs 1-2 route the add via ScalarE-evac + GpSimd (keeps the
            # DVE queue clear so gs casts land promptly); the last pair keeps
            # the single DVE add so the drain chain stays short.
            if bt >= 2 and (bt - 1) % 2 == 1:
                k = (bt - 1) // 2
                sl = r_sb[:, 2 * k:2 * k + 2, :]
                if p == 0:
                    nc.scalar.copy(out=sl, in_=rp[k])
                elif not drain:
                    rps = snap.tile([P, 2, NF], F32, tag="rps")
                    nc.scalar.copy(out=rps, in_=rp[k])
                    nc.gpsimd.tensor_add(out=sl, in0=rps, in1=sl)
                else:
                    nc.vector.tensor_add(out=sl, in0=rp[k], in1=sl)
                if drain:
                    if k < 3:
                        nc.sync.dma_start(out=out_d[:, 2 * k:2 * k + 2, :],
                                          in_=sl)
                    else:
                        # final chunk partition-split across both HW queues:
                        # halves the post-last-matmul descgen+transfer tail
                        nc.sync.dma_start(out=out_d[0:HP, 6:8, :],
                                          in_=sl[0:HP])
                        nc.scalar.dma_start(out=out_d[HP:P, 6:8, :],
                                            in_=sl[HP:P])
            if bt < TB:
                # [D] prep groups, interleaved as PE filler
                for _ in range(drate):
                    if nthunks:
                        nthunks.pop(0)()
                # [E]+[F] mask the diag pair, then its r contribution
                if bt % 2 == 1:
                    omd = snap.tile([P, 4, P], BF16, tag="omd")
                    nc.vector.tensor_mul(omd, pwd, mask4)
                    for b2 in (bt - 1, bt):
                        k = b2 // 2
                        if rp[k] is None:
                            rp[k] = prp.tile([P, 2, NF], F32, tag="rp",
                                             name=f"rp{k}")
                        for hh in range(2):
                            nc.tensor.matmul(
                                rp[k][:, b2 % 2, :],
                                lhsT=omd[:, 2 * (b2 % 2) + hh, :],
                                rhs=es2[:, b2, hh * NF:(hh + 1) * NF],
                                start=(not rp_started[k]),
                                stop=False,
                                skip_group_check=True,
                            )
                            rp_started[k] = True
            gs_prev2 = gs_prev
            gs_prev = gs
            stro_prev = stro

    # pair-0 prep runs bare during the DMA ramp; rotate over all three
    # transient PSUM pools so evacuation latency never blocks the PE. The
    # deferred bulk DMAs are emitted between groups so the scalar/gpsimd
    # queues enqueue them only once the critical transfers are in flight.
    qsT2, es2, thunks = prep_groups(
        0, pool_cycle=[(pwp, "pw"), (pdp, "pwd"), (prp, "rp")])
    for th in thunks:
        th()
    carry = []   # pair-3 prep groups deferred into pair 3's own body as
    # PE filler (it has no next-pair prep to hide the gs-copy latency behind)
    for p in range(NPAIR):
        if p + 1 < NPAIR:
            nqsT2, nes2, nthunks = prep_groups(
                p + 1, alt_from=(7 if p + 1 == NPAIR - 1 else None))
            if p + 1 == NPAIR - 1:
                nthunks, carry = nthunks[:7], nthunks[7:]
        else:
            nqsT2, nes2, nthunks = None, None, carry
        body(p, qsT2, es2, nthunks, drain=(p == NPAIR - 1),
             drate=(1 if p == NPAIR - 2 else 2),
             stripes=(p == NPAIR - 1))
        for th in nthunks:   # any leftovers
            th()
        qsT2, es2 = nqsT2, nes2


def build():
    from contextlib import ExitStack

    nc = bacc.Bacc(
        "TRN2",
        target_bir_lowering=False,
        debug=False,
        enable_asserts=False,
        num_devices=NCORES,
    )
    in0_d = nc.dram_tensor("in0", [P, 3072], BF16, kind="ExternalInput").ap()
    inE_d = nc.dram_tensor("inE", [P, 1024], BF16, kind="ExternalInput").ap()
    in2_d = nc.dram_tensor("in2", [P, 8192], BF16, kind="ExternalInput").ap()
    out_d = nc.dram_tensor("out", [P, TB, NF], F32, kind="ExternalOutput").ap()
    with tile.TileContext(nc) as tc:
        with ExitStack() as ctx:
            _emit(tc, nc, in0_d, inE_d, in2_d, out_d, ctx)
    nc.compile()
    return nc


_NC = None


def _get_nc():
    global _NC
    if _NC is None:
        _NC = build()
    return _NC


def _in_maps(s, Q, E):
    bf = ml_dtypes.bfloat16
    s = np.asarray(s, np.float32)
    Qf = np.asarray(Q, np.float32)
    Ef = np.asarray(E, np.float32)
    # Q_d[i1, h*2+ic, j] = Q[h, ic*128+i1, j]
    Qd = np.ascontiguousarray(
        Qf.reshape(H, JC, P, NF).transpose(2, 0, 1, 3).reshape(P, H * JC, NF)
    ).astype(bf)
    # ETd[j1, pair, jc, hh, i] = E[2*pair+hh, i, jc*128+j1]  (pair-major)
    ETd = np.ascontiguousarray(
        Ef.transpose(2, 0, 1)                  # [j, h, i]
        .reshape(JC, P, NPAIR, 2, NF)          # [jc, j1, pair, hh, i]
        .transpose(1, 2, 0, 3, 4)              # [j1, pair, jc, hh, i]
    ).astype(bf)
    maps = []
    for b in range(B):
        sb = s[b]
        sd = np.ascontiguousarray(
            sb.reshape(TB, P, NF).transpose(1, 0, 2)).astype(bf)
        sTd = np.ascontiguousarray(
            sb.T.reshape(JC, P, 2, 512).transpose(1, 2, 0, 3)).astype(bf)
        in0 = np.concatenate([
            sTd.reshape(P, 2048),
            Qd[:, 0:4].reshape(P, 1024),
        ], axis=1)
        in2 = np.concatenate([
            sd.reshape(P, 2048),
            Qd[:, 4:16].reshape(P, 3072),
            ETd[:, 1:4].reshape(P, 3072),
        ], axis=1)
        maps.append({"in0": np.ascontiguousarray(in0),
                     "inE": np.ascontiguousarray(ETd[:, 0].reshape(P, 1024)),
                     "in2": np.ascontiguousarray(in2)})
    return maps


def _unpack(res):
    return np.stack([
        np.ascontiguousarray(
            res.results[b]["out"].transpose(1, 0, 2).reshape(T, NF))
        for b in range(B)], axis=0)


def kernel(s, Q, E):
    nc = _get_nc()
    res = run_bass_kernel_spmd(
        nc, _in_maps(s, Q, E), core_ids=list(range(NCORES)))
    return _unpack(res)


def _register_ntff_hook():
    """Register the axon NTFF profile hook trn_boot skips (the agent image's
    antenv lacks axon_hooks); best-effort — profiling-only."""
    import sys
    import types
    try:
        import antenv
        if "antenv.axon_hooks" in sys.modules:
            return
        mod = types.ModuleType("antenv.axon_hooks")
        _state = {"hook": None}
        mod.set_axon_ntff_profile_hook = lambda h: _state.__setitem__("hook", h)
        mod.get_axon_ntff_profile_hook = lambda: _state["hook"]
        sys.modules["antenv.axon_hooks"] = mod
        antenv.axon_hooks = mod
        from trn_agent_boot.trn_boot import _ntff_profile_via_ctypes
        mod.set_axon_ntff_profile_hook(
            _ntff_profile_via_ctypes("/opt/axon/libaxon_pjrt.so"))
    except Exception:
        pass


def run_profiled(s, Q, E, tmpdir=None):
    _register_ntff_hook()
    nc = _get_nc()
    res = run_bass_kernel_spmd(
        nc, _in_maps(s, Q, E), core_ids=list(range(NCORES)),
        trace=True, tmpdir=tmpdir)
    return _unpack(res), res.exec_time_ns



# revision 42
# speedup vs baseline: 1.1009x; 1.0011x over previous
"""Trainium2 Bass kernel (v10) for nn_AttentionBlock — reassociated causal attention.

Reference (per batch b):
    qs[t,j]    = sum_i s[t,i] Q[h,i,j]
    Omega[t,u] = sum_j qs[t,j] s[u,j]       (causal: keep u <= t)
    es[u,i]    = sum_j E[h,i,j] s[u,j]
    r[t,i]     = sum_h sum_u Omega[t,u] es[u,i]

Reassociation: for full (below-diagonal) 128-token blocks,
    sum_{u in blk} Omega[t,u] es[u,i] = qs[t,:] @ (s[blk].T @ es[blk])
so r's off-diagonal part = qs[bt] @ Gsum(bt) with Gsum the PSUM-accumulated
prefix of G_uc = s[uc].T @ es[uc]; only diagonal 128x128 Omega blocks are
materialized, masked by a DVE multiply with a precomputed triangular mask.

v22 over v9: heads processed in PAIRS with concatenated rhs operands (es, G,
and diag-Omega matmuls run at N=512/N=256 over both heads — 422 vs 601 PE
instructions), r accumulated in SBUF f32 via per-chunk DVE adds (frees 4 PSUM
banks for deeper transient rotation and removes the end-of-kernel PSUM
drain; pairs 1-2 route the per-chunk add ScalarE->SBUF then sum on the
otherwise-idle GpSimd so the DVE queue never delays a Gsum cast; the last
pair keeps a single DVE add so the output-drain chain stays short), the
Gsum snapshot is consumed one bt-iteration late (software
pipeline; the r_off matmuls are emitted BEFORE the G-accumulate so the PE
FIFO transitively orders each snapshot read before the next accumulate —
Tile does not emit that WAR edge itself), all DRAM tensors are pre-swizzled
on the host into [128, contiguous] layouts (12 large 2KB+/partition-line
DMAs at full HBM bandwidth), a burst of dummy matmuls at t=0 warms the PE
HAM clock gate during the input-DMA ramp, and the last head-pair (which has
no next-pair prep) gets 9 of its own prep groups deferred into its body as
PE filler PLUS a "stripe" formulation (r_off(tb) = qs[tb] @ S[tb-1] + an
explicit full Omega[tb,tb-1] block) that consumes each Gsum snapshot two
iterations after capture — the copy latency can no longer stall the PE and
the HAM clock gate stays at 8/8 through the whole tail (an explicit
ldweights on the snapshot guards the read-before-next-accumulate order).

v31 over v22 (trace-driven; balanced A/B: 96.1us mean vs 97.9 for v22,
best 95.05): (1) DMA descriptor GENERATION is the serial fabric resource
(~69 desc/us shared across all queues, one descriptor per dst partition
line; a DMA's transfer starts only after its full descgen; partition-split
halves run at HALF rate) — the 12-DMA input wave is repacked HOST-SIDE
into three FIFO DMAs on one queue: [sT|Qp0] 768KB 6KB lines (~10.3us),
[ETp0] (~11.8), then the 2MB bulk [s|Q4:16|ET1:4] 16KB lines (~17).
Warmups 12->6. (2) Gsum snapshot halves are SEPARATE tiles (a shared tile
tied the DVE cast to the ScalarE cast of 3 snapshots ago via whole-tile
WAW tracking, head-of-line blocking the DVE queue). (3) Final output chunk
partition-split across both HW queues (halves the post-last-matmul tail).
Body schedule is v22's unchanged — rebalancing experiments (carry removal,
prp/pdp rebanking, mask-mul offload, chained per-tensor input DMAs) all
lost more in new WAR/queue/arrival stalls than they saved. GpSimd (Pool)
has NO PSUM access on TRN2, so every PSUM evacuation shares DVE+ScalarE
(~85% busy through the body). Further chaining of inE/in2 issues (v32/33)
delayed s past its [B]@bt1 deadline, and folding ETp0 into in0 with in2
chained behind (v34) just moved the same ~2us stall from ETp0-arrival to
s-arrival at body start — the transfer-time budget before [B]@bt1 (~20us)
fits the critical 1MB + bulk 2MB only with the 3-DMA split as-is; the
scheduler's issue order for it is benign in practice; leave it unchained.

Gotcha encoded here: a PSUM accumulation group spanning N banks needs
start=True on EACH bank's first matmul (start clears has_written for one
bank only).

Distribution: data-parallel over batch (8 batches = 8 cores, no collectives).
All matmuls bf16; f32 PSUM accumulation. ~98.7us vs 111us baseline; rel err
3.07e-3. Note: back-to-back benchmarking runs heat the part into a lower
power state (~+20% exec time); space measurement runs out.
"""

import numpy as np
import ml_dtypes

import concourse.bacc as bacc
import concourse.mybir as mybir
import concourse.tile as tile
from concourse.bass_utils import run_bass_kernel_spmd

B = 8      # batch (== number of cores)
T = 1024   # tokens
NF = 256   # feature dim n
H = 8      # heads
P = 128    # partitions
TB = T // P    # 8 token blocks
JC = NF // P   # 2 feature chunks
NPAIR = H // 2
NCORES = 8

F32 = mybir.dt.float32
BF16 = mybir.dt.bfloat16
IS_GE = mybir.AluOpType.is_ge


def _emit(tc, nc, in0_d, inE_d, in2_d, out_d, ctx):
    res = ctx.enter_context(tc.tile_pool(name="res", bufs=1))
    work = ctx.enter_context(tc.tile_pool(name="work", bufs=2))
    snap = ctx.enter_context(tc.tile_pool(name="snap", bufs=3))
    prp = ctx.enter_context(tc.tile_pool(name="prp", bufs=1, space="PSUM"))
    pgp = ctx.enter_context(tc.tile_pool(name="pgp", bufs=1, space="PSUM"))
    pwp = ctx.enter_context(tc.tile_pool(name="pwp", bufs=2, space="PSUM"))
    pdp = ctx.enter_context(tc.tile_pool(name="pdp", bufs=3, space="PSUM"))

    # Host-packed inputs: DMA descriptor generation is the serial resource
    # (~69 desc/us shared across queues, one desc per dst partition line;
    # transfers start only after a DMA's full descgen; per-core bw ~400GB/s),
    # so the wave is three FIFO DMAs on one queue: critical [sT|Qp0] 768KB
    # (done ~10.3us), ETp0 (~11.8), then the 2MB bulk [s|Q4:16|ET1:4] (~17).
    # Partition-splits/per-tensor DMAs only multiply descriptors (a
    # 64-partition half runs at HALF rate).
    in0_sb = res.tile([P, 3072], BF16)   # [ sT(2048) | Qp0(1024) ]
    inE_sb = res.tile([P, 1024], BF16)   # [ ETp0 ]
    in2_sb = res.tile([P, 8192], BF16)   # [ s(2048) | Q[4:16](3072) | ET[1:4](3072) ]
    mask4 = res.tile([P, 4, P], BF16)    # [u, (bt%2, h), t]: 1 where u <= t
    warm = res.tile([P, 640], BF16)
    r_sb = res.tile([P, TB, NF], F32)    # final r accumulator (SBUF)

    HP = P // 2
    nc.scalar.dma_start(out=in0_sb, in_=in0_d)
    nc.scalar.dma_start(out=inE_sb, in_=inE_d)
    nc.scalar.dma_start(out=in2_sb, in_=in2_d)

    # views into the packed tiles (old sT_sb / Q_sb / ET_sb / s_sb layouts)
    def sTv(tcx, jc, lo=0, hi=512):
        base = (tcx * 2 + jc) * 512
        return in0_sb[:, base + lo:base + hi]

    def Qv(c, lo=0, hi=NF):
        if c < 4:
            return in0_sb[:, 2048 + c * NF + lo:2048 + c * NF + hi]
        return in2_sb[:, 2048 + (c - 4) * NF + lo:2048 + (c - 4) * NF + hi]

    def ETv(p, jc):
        if p == 0:
            return inE_sb[:, jc * 512:jc * 512 + 512]
        base = 5120 + (p - 1) * 1024 + jc * 512
        return in2_sb[:, base:base + 512]

    def sv(uc, jc):
        base = uc * NF + jc * P
        return in2_sb[:, base:base + P]

    # warm tile zeroed on the (idle) vector engine so warmup matmuls are not
    # queued behind gpsimd's DMA issues; mask setup stays on gpsimd.
    nc.vector.memset(warm, 0.0)
    nc.gpsimd.memset(mask4, 1.0)
    nc.gpsimd.affine_select(
        out=mask4, in_=mask4,
        pattern=[[0, 4], [1, P]],
        compare_op=IS_GE,   # keep 1.0 where t - u >= 0, else 0
        fill=0.0, base=0, channel_multiplier=-1,
    )

    # HAM warmup: dummy matmuls on a zeroed tile while input DMAs run, so the
    # PE clock gate is at 8/8 by the time real matmuls start.
    pwarm = pdp.tile([P, 512], F32, tag="pwd", name="pwarm")
    for _ in range(6):
        nc.tensor.matmul(pwarm, lhsT=warm[:, 0:128], rhs=warm[:, 128:640],
                         start=True, stop=True, skip_group_check=True)

    # Prep evacuations alternate VectorE/ScalarE.
    movers = [nc.vector.tensor_copy, nc.scalar.copy]
    mv = [0]

    def mover(out, in_, alt=True):
        movers[mv[0] % 2](out=out, in_=in_)
        mv[0] += 1

    # ---- per-pair prep: qsT for both heads and pair-concatenated es
    def prep_groups(p, pool_cycle=None, alt_from=None):
        h0 = 2 * p
        qsT2 = work.tile([P, 2, JC, T], BF16, tag="qsT", name=f"qsT{p}")
        es2 = work.tile([P, TB, 2 * NF], BF16, tag="es", name=f"es{p}")
        base_alt = pool_cycle is not None
        pool_cycle = pool_cycle or [(pwp, "pw")]

        def qsT_group(hh, jc, tcx, pool, tag, alt):
            pw = pool.tile([P, 512], F32, tag=tag, name="pwq")
            for ic in range(JC):
                nc.tensor.matmul(
                    pw,
                    lhsT=Qv((h0 + hh) * 2 + ic, jc * P, (jc + 1) * P),
                    rhs=sTv(tcx, ic),
                    start=(ic == 0), stop=(ic == JC - 1),
                    skip_group_check=True,
                )
            mover(qsT2[:, hh, jc, tcx * 512:(tcx + 1) * 512], pw, alt=alt)

        def es_group(uc, pool, tag, alt):
            pw = pool.tile([P, 512], F32, tag=tag, name="pwe")
            for jc in range(JC):
                nc.tensor.matmul(
                    pw,
                    lhsT=sTv(uc // 4, jc, (uc % 4) * P, (uc % 4 + 1) * P),
                    rhs=ETv(h0 // 2, jc),
                    start=(jc == 0), stop=(jc == JC - 1),
                    skip_group_check=True,
                )
            mover(es2[:, uc, :], pw, alt=alt)

        # Order groups progressively: tcx0-qsT and low-uc es first (their
        # DMAs land first in the prologue); the tail of the list is safe to
        # defer into the consuming pair's own body (late-uc es / tcx1 qsT).
        specs = []
        for hh in range(2):
            for jc in range(JC):
                specs.append(("q", (hh, jc, 0)))
        for uc in range(3):
            specs.append(("e", (uc,)))
        for hh in range(2):
            for jc in range(JC):
                specs.append(("q", (hh, jc, 1)))
        for uc in range(3, TB):
            specs.append(("e", (uc,)))
        thunks = []
        for i, (kind, args) in enumerate(specs):
            pool, tag = pool_cycle[i % len(pool_cycle)]
            alt = base_alt or (alt_from is not None and i >= alt_from)
            if kind == "q":
                thunks.append(
                    lambda a=args, pool=pool, tag=tag, alt=alt:
                    qsT_group(*a, pool, tag, alt))
            else:
                thunks.append(
                    lambda a=args, pool=pool, tag=tag, alt=alt:
                    es_group(*a, pool, tag, alt))
        return qsT2, es2, thunks

    def body(p, qsT2, es2, nthunks, drain, drate=2, stripes=False):
        # stripes=True (last pair): r_off(tb) = qs[tb] @ S[tb-1] + explicit
        # Omega[tb, tb-1] stripe, so each Gsum snapshot is consumed TWO
        # iterations after it is taken — the PSUM->SBUF cast latency can
        # never stall the PE even with no next-pair prep to hide it behind.
        pg2 = pgp.tile([P, JC, 512], F32, tag="pg", name=f"pg{p}")
        gs_prev = None
        gs_prev2 = None
        gs = None
        pwd = None
        omd = None
        stro = None
        stro_prev = None
        rp = [None] * 4        # rp chunk tiles, one per 2-bt
        rp_started = [False] * 4
        for bt in range(TB + 1):
            if bt < TB:
                # [A] diag OmegaT block for both heads: [u, (h, t)]
                if bt % 2 == 0:
                    pwd = pdp.tile([P, 4, P], F32, tag="pwd", name="pwd")
                for jc in range(JC):
                    nc.tensor.matmul(
                        pwd[:, 2 * (bt % 2):2 * (bt % 2) + 2, :],
                        lhsT=sTv(bt // 4, jc, (bt % 4) * P, (bt % 4 + 1) * P),
                        rhs=qsT2[:, :, jc, bt * P:(bt + 1) * P],
                        start=(bt % 2 == 0 and jc == 0),
                        stop=(bt % 2 == 1 and jc == JC - 1),
                        skip_group_check=True,
                    )
                if stripes and bt >= 1:
                    # stripe OmegaT[u in bt-1, (h, t in bt)] (full block, no
                    # mask); consumed by [G] next iteration
                    pstro = pdp.tile([P, 4, P], F32, tag="pwd", name="pstro")
                    for jc in range(JC):
                        nc.tensor.matmul(
                            pstro[:, 0:2, :],
                            lhsT=sTv((bt - 1) // 4, jc, ((bt - 1) % 4) * P,
                                     ((bt - 1) % 4 + 1) * P),
                            rhs=qsT2[:, :, jc, bt * P:(bt + 1) * P],
                            start=(jc == 0), stop=(jc == JC - 1),
                            skip_group_check=True,
                        )
                    stro = snap.tile([P, 2, P], BF16, tag="stro")
                    nc.scalar.copy(out=stro, in_=pstro[:, 0:2, :])
            # [G] r_off(bt-1) = qs[bt-1] @ Gsum[bt-1] via last iter's snapshot.
            # MUST be emitted before [B]: [G]'s wait on the gs copy transitively
            # (via the PE FIFO) keeps this iteration's pg2-accumulating matmuls
            # from racing ahead of last iteration's snapshot read.
            if bt >= 2:
                tb = bt - 1
                k = tb // 2
                if rp[k] is None:
                    rp[k] = prp.tile([P, 2, NF], F32, tag="rp", name=f"rp{k}")
                if stripes:
                    for hh in range(2):
                        nc.tensor.matmul(
                            rp[k][:, tb % 2, :],
                            lhsT=stro_prev[:, hh, :],
                            rhs=es2[:, tb - 1, hh * NF:(hh + 1) * NF],
                            start=(not rp_started[k]),
                            stop=(tb == 1 and hh == 1),
                            skip_group_check=True,
                        )
                        rp_started[k] = True
                if not stripes or tb >= 2:
                    gsrc = gs_prev2 if stripes else gs_prev
                    for hh in range(2):
                        for jc in range(JC):
                            nc.tensor.matmul(
                                rp[k][:, tb % 2, :],
                                lhsT=qsT2[:, hh, jc, tb * P:(tb + 1) * P],
                                rhs=gsrc[jc][:, hh * NF:(hh + 1) * NF],
                                start=(not rp_started[k]),
                                stop=(tb % 2 == 1 and hh == 1 and jc == JC - 1),
                                skip_group_check=True,
                            )
                            rp_started[k] = True
            if bt < TB:
                # [B] Gsum prefix accumulation + [C] snapshot. Under stripes
                # S[7] is never consumed: skip the last accumulate+snapshot.
                if bt >= 1 and not (stripes and bt == TB - 1):
                    uc = bt - 1
                    if stripes and gs_prev is not None:
                        # PE-FIFO guard: orders this iteration's accumulate
                        # after last iteration's snapshot read (Tile emits no
                        # WAR edge for mid-group PSUM reads).
                        nc.tensor.ldweights(weights=gs_prev[0][:, 0:P])
                    for jc in range(JC):
                        nc.tensor.matmul(
                            pg2[:, jc, :],
                            lhsT=sv(uc, jc),
                            rhs=es2[:, uc, :],
                            # pg2 spans two banks (one per jc): each bank's
                            # first matmul needs start=True to clear its own
                            # has_written bits (start only clears ONE bank).
                            start=(bt == 1),
                            stop=(bt == (TB - 2 if stripes else TB - 1)
                                  and jc == JC - 1),
                            skip_group_check=True,
                        )
                    # gs halves are SEPARATE tiles: one shared tile ties
                    # the DVE cast to the ScalarE cast of 3 snapshots ago
                    # via whole-tile WAW tracking, head-of-line blocking the
                    # DVE queue behind a busy ScalarE
                    gs = (snap.tile([P, 512], BF16, tag="gs0", name="gs0"),
                          snap.tile([P, 512], BF16, tag="gs1", name="gs1"))
                    nc.vector.tensor_copy(out=gs[0], in_=pg2[:, 0])
                    nc.scalar.copy(out=gs[1], in_=pg2[:, 1])
            # [H] chunk complete -> accumulate into SBUF r, drain if last.
            # Pairs 1-2 route the add via ScalarE-evac + GpSimd (keeps the
            # DVE queue clear so gs casts land promptly); the last pair keeps
            # the single DVE add so the drain chain stays short.
            if bt >= 2 and (bt - 1) % 2 == 1:
                k = (bt - 1) // 2
                sl = r_sb[:, 2 * k:2 * k + 2, :]
                if p == 0:
                    nc.scalar.copy(out=sl, in_=rp[k])
                elif not drain:
                    rps = snap.tile([P, 2, NF], F32, tag="rps")
                    nc.scalar.copy(out=rps, in_=rp[k])
                    nc.gpsimd.tensor_add(out=sl, in0=rps, in1=sl)
                else:
                    nc.vector.tensor_add(out=sl, in0=rp[k], in1=sl)
                if drain:
                    if k < 3:
                        nc.sync.dma_start(out=out_d[:, 2 * k:2 * k + 2, :],
                                          in_=sl)
                    else:
                        # final chunk partition-split across both HW queues:
                        # halves the post-last-matmul descgen+transfer tail
                        nc.sync.dma_start(out=out_d[0:HP, 6:8, :],
                                          in_=sl[0:HP])
                        nc.scalar.dma_start(out=out_d[HP:P, 6:8, :],
                                            in_=sl[HP:P])
            if bt < TB:
                # [D] prep groups, interleaved as PE filler
                for _ in range(drate):
                    if nthunks:
                        nthunks.pop(0)()
                # [E]+[F] mask the diag pair, then its r contribution
                if bt % 2 == 1:
                    omd = snap.tile([P, 4, P], BF16, tag="omd")
                    nc.vector.tensor_mul(omd, pwd, mask4)
                    for b2 in (bt - 1, bt):
                        k = b2 // 2
                        if rp[k] is None:
                            rp[k] = prp.tile([P, 2, NF], F32, tag="rp",
                                             name=f"rp{k}")
                        for hh in range(2):
                            nc.tensor.matmul(
                                rp[k][:, b2 % 2, :],
                                lhsT=omd[:, 2 * (b2 % 2) + hh, :],
                                rhs=es2[:, b2, hh * NF:(hh + 1) * NF],
                                start=(not rp_started[k]),
                                stop=False,
                                skip_group_check=True,
                            )
                            rp_started[k] = True
            gs_prev2 = gs_prev
            gs_prev = gs
            stro_prev = stro

    # pair-0 prep runs bare during the DMA ramp; rotate over all three
    # transient PSUM pools so evacuation latency never blocks the PE. The
    # deferred bulk DMAs are emitted between groups so the scalar/gpsimd
    # queues enqueue them only once the critical transfers are in flight.
    qsT2, es2, thunks = prep_groups(
        0, pool_cycle=[(pwp, "pw"), (pdp, "pwd"), (prp, "rp")])
    for th in thunks:
        th()
    carry = []   # pair-3 prep groups deferred into pair 3's own body as
    # PE filler (it has no next-pair prep to hide the gs-copy latency behind)
    for p in range(NPAIR):
        if p + 1 < NPAIR:
            nqsT2, nes2, nthunks = prep_groups(
                p + 1, alt_from=(7 if p + 1 == NPAIR - 1 else None))
            if p + 1 == NPAIR - 1:
                nthunks, carry = nthunks[:7], nthunks[7:]
        else:
            nqsT2, nes2, nthunks = None, None, carry
        body(p, qsT2, es2, nthunks, drain=(p == NPAIR - 1),
             drate=(1 if p == NPAIR - 2 else 2),
             stripes=(p == NPAIR - 1))
        for th in nthunks:   # any leftovers
            th()
        qsT2, es2 = nqsT2, nes2


def build():
    from contextlib import ExitStack

    nc = bacc.Bacc(
        "TRN2",
        target_bir_lowering=False,
        debug=False,
        enable_asserts=False,
        num_devices=NCORES,
    )
    in0_d = nc.dram_tensor("in0", [P, 3072], BF16, kind="ExternalInput").ap()
    inE_d = nc.dram_tensor("inE", [P, 1024], BF16, kind="ExternalInput").ap()
    in2_d = nc.dram_tensor("in2", [P, 8192], BF16, kind="ExternalInput").ap()
    out_d = nc.dram_tensor("out", [P, TB, NF], F32, kind="ExternalOutput").ap()
    with tile.TileContext(nc) as tc:
        with ExitStack() as ctx:
            _emit(tc, nc, in0_d, inE_d, in2_d, out_d, ctx)
    nc.compile()
    return nc


_NC = None


def _get_nc():
    global _NC
    if _NC is None:
        _NC = build()
    return _NC


def _in_maps(s, Q, E):
    bf = ml_dtypes.bfloat16
    s = np.asarray(s, np.float32)
    Qf = np.asarray(Q, np.float32)
    Ef = np.asarray(E, np.float32)
    Qd = np.ascontiguousarray(
        Qf.reshape(H, JC, P, NF).transpose(2, 0, 1, 3).reshape(P, H * JC, NF)
    ).astype(bf)
    # ETd[j1, pair, jc, hh, i] = E[2*pair+hh, i, jc*128+j1]  (pair-major)
    ETd = np.ascontiguousarray(
        Ef.transpose(2, 0, 1).reshape(JC, P, NPAIR, 2, NF)
        .transpose(1, 2, 0, 3, 4)).astype(bf)
    maps = []
    for b in range(B):
        sb = s[b]
        sd = np.ascontiguousarray(
            sb.reshape(TB, P, NF).transpose(1, 0, 2)).astype(bf)
        sTd = np.ascontiguousarray(
            sb.T.reshape(JC, P, 2, 512).transpose(1, 2, 0, 3)).astype(bf)
        in0 = np.concatenate([sTd.reshape(P, 2048),
                              Qd[:, 0:4].reshape(P, 1024)], axis=1)
        in2 = np.concatenate([sd.reshape(P, 2048),
                              Qd[:, 4:16].reshape(P, 3072),
                              ETd[:, 1:4].reshape(P, 3072)], axis=1)
        maps.append({"in0": np.ascontiguousarray(in0),
                     "inE": np.ascontiguousarray(ETd[:, 0].reshape(P, 1024)),
                     "in2": np.ascontiguousarray(in2)})
    return maps


def _unpack(res):
    return np.stack([
        np.ascontiguousarray(
            res.results[b]["out"].transpose(1, 0, 2).reshape(T, NF))
        for b in range(B)], axis=0)


def kernel(s, Q, E):
    nc = _get_nc()
    res = run_bass_kernel_spmd(
        nc, _in_maps(s, Q, E), core_ids=list(range(NCORES)))
    return _unpack(res)


def _register_ntff_hook():
    """Register the axon NTFF profile hook trn_boot skips; profiling-only."""
    import sys
    import types
    try:
        import antenv
        if "antenv.axon_hooks" in sys.modules:
            return
        mod = types.ModuleType("antenv.axon_hooks")
        _state = {"hook": None}
        mod.set_axon_ntff_profile_hook = lambda h: _state.__setitem__("hook", h)
        mod.get_axon_ntff_profile_hook = lambda: _state["hook"]
        sys.modules["antenv.axon_hooks"] = mod
        antenv.axon_hooks = mod
        from trn_agent_boot.trn_boot import _ntff_profile_via_ctypes
        mod.set_axon_ntff_profile_hook(
            _ntff_profile_via_ctypes("/opt/axon/libaxon_pjrt.so"))
    except Exception:
        pass


def run_profiled(s, Q, E, tmpdir=None):
    _register_ntff_hook()
    nc = _get_nc()
    res = run_bass_kernel_spmd(
        nc, _in_maps(s, Q, E), core_ids=list(range(NCORES)),
        trace=True, tmpdir=tmpdir)
    return _unpack(res), res.exec_time_ns

